# revision 7
# baseline (speedup 1.0000x reference)
"""Trainium2 Bass kernel for nn_CrossAttention (4-layer MLP -> cross-attention).

Sharding: data-parallel across batch B=8, one batch element per NeuronCore.

v2 strategy — all-fp8 DoubleRow tensor work (PE was 92% busy in v1 with
~45% of its cycles in bf16):

  - Wq/Wk folded on host: M = Wq @ Wk^T, so scores = h @ M @ y^T and the
    k-projection disappears. The softmax-row-constant term h.(Wq bk)
    cancels; the column term c_j = y_j.(Wk bq) is a per-kv-partition bias
    applied for free inside the exp activation.
  - attention uses E@v = Sv + (E-1)@v: (E-1) in [-0.12, 0.12] quantizes to
    fp8 with ~16x less absolute error than E itself, and fp8 v errors only
    enter scaled by (E-1). Sv = (sum_k y_k) @ Wv is computed exactly-enough
    in bf16 (tiny matvec) and broadcast via a DRAM bounce. bv is added
    exactly in f32 at the very end: out = (po + Sv0)*rinv + bv.
  - v projection also runs fp8 DR from the already-resident y8 tiles.
  - M8 prescaled x32 on host (fp8 subnormal avoidance), compensated in the
    exp activation scale; col8 prescaled x8 likewise.

Remaining bf16: the 16 Sv matvec matmuls. All accumulation fp32 in PSUM.

fp8 operands are pair-packed for DoubleRow: logical contraction index
k = (2t+r)*128+p lives in tile t, partition p, middle index r, i.e. SBUF
tiles [128, 2, N] (pre-packed on host to [K/2, 2*N] per tile DMA).

Everything is SBUF-resident; y-side and attention operands prefetch early.
"""

import sys

if "/opt/trn_rl_repo" not in sys.path:
    sys.path.insert(0, "/opt/trn_rl_repo")

import numpy as np
import ml_dtypes

P = 128
D = 1024
DB = 512
S = 2048
KD = D // P       # 8 feature tiles of 128
KB = DB // P      # 4
PD = KD // 2      # 4 fp8 pair-tiles for a 1024 contraction
PB = KB // 2      # 2 for 512
NT = S // P       # 16 token tiles
NP = NT // 2      # 8 kv pair-tiles
NB = 512          # moving-operand free-dim block
NBLK = S // NB    # 4 token blocks
HALF = S // 2     # q processed in 2 halves during attention
NCORES = 8
SCALE = float(1.0 / np.sqrt(D))
MS = 32.0         # host prescale of M8 (fp8 subnormal avoidance)
CS = 8.0          # host prescale of col8

BF16 = ml_dtypes.bfloat16
FP8 = ml_dtypes.float8_e4m3

_NC = None


def build_nc():
    """Build + compile the per-core Bass program (cached)."""
    global _NC
    if _NC is not None:
        return _NC

    from contextlib import ExitStack
    import concourse.bass as bass
    import concourse.tile as tile
    from concourse import bacc, mybir

    BF = mybir.dt.bfloat16
    F8 = mybir.dt.float8e4
    F32 = mybir.dt.float32
    AF = mybir.ActivationFunctionType
    ALU = mybir.AluOpType
    DR = mybir.MatmulPerfMode.DoubleRow

    nc = bacc.Bacc("TRN2", target_bir_lowering=False, debug=False,
                   num_devices=NCORES)

    def din(name, shape, dt):
        return nc.dram_tensor(name, shape, dt, kind="ExternalInput").ap()

    # fp8 operands arrive pair-packed: [K/2, 2*N]
    x8d = din("x8", [D // 2, 2 * S], F8)
    y8d = din("y8", [D // 2, 2 * S], F8)
    yTd = din("yT", [D, S], BF)
    W1d = din("W1", [D // 2, 2 * D], F8)
    W2d = din("W2", [D // 2, 2 * DB], F8)
    W3d = din("W3", [DB // 2, 2 * D], F8)
    W4d = din("W4", [D // 2, 2 * D], F8)
    M8d = din("M8", [D // 2, 2 * D], F8)
    C8d = din("C8", [D // 2, 2 * 16], F8)
    Wv8d = din("Wv8", [D // 2, 2 * D], F8)
    Wvbd = din("Wvb", [D, D], BF)
    b1 = din("b1", [P, KD], F32)
    b2 = din("b2", [P, KB], F32)
    b3 = din("b3", [P, KD], F32)
    b4 = din("b4", [P, KD], F32)
    bv = din("bv", [D], F32)
    out = nc.dram_tensor("out", [S, D], F32, kind="ExternalOutput").ap()

    with tile.TileContext(nc) as tc, ExitStack() as ctx:
        small = ctx.enter_context(tc.tile_pool(name="small", bufs=1))
        rpool = ctx.enter_context(tc.tile_pool(name="rpool", bufs=4))
        outp = ctx.enter_context(tc.tile_pool(name="outp", bufs=2))

        def load_bias(src, cols, tag):
            t = small.tile([P, cols], F32, tag=tag, name=tag)
            nc.gpsimd.dma_start(out=t, in_=src)
            return t

        b1_sb = load_bias(b1, KD, "b1")
        b2_sb = load_bias(b2, KB, "b2")
        b3_sb = load_bias(b3, KD, "b3")
        b4_sb = load_bias(b4, KD, "b4")

        # bv replicated across partitions for the final (exact, fp32) bias add
        bv_rep = small.tile([P, D], F32, tag="bvrep", name="bvrep")
        bv_bcast = bass.AP(tensor=bv.tensor, offset=bv.offset,
                           ap=[[0, P]] + list(bv.ap))
        nc.gpsimd.dma_start(out=bv_rep, in_=bv_bcast)

        # fp8 all-ones moving operand for the denominator matmuls
        ones8 = small.tile([P, 2, 16], F8, tag="ones8", name="ones8")
        nc.vector.memset(ones8, 1.0)
        # per-partition -1.0 bias for the (E-1) activation
        negone = small.tile([P, 1], F32, tag="negone", name="negone")
        nc.vector.memset(negone, -1.0)

        # Sv0 replicated (filled in stage B via DRAM bounce)
        sv_rep = small.tile([P, D], F32, tag="svrep", name="svrep")
        # exp bias: SCALE * c_j per kv token, [128, 16] (col tk)
        cbias = small.tile([P, NT], F32, tag="cbias", name="cbias")

        def alloc_pairs(pool, pairs, n, tag, dt=F8):
            """fp8 pair-packed tiles [P, 2, n]."""
            return [pool.tile([P, 2, n], dt, tag=f"{tag}{t}", name=f"{tag}{t}")
                    for t in range(pairs)]

        def load_pairs(tiles, src, n):
            for t, tl in enumerate(tiles):
                nc.sync.dma_start(
                    out=tl,
                    in_=src[t * P:(t + 1) * P, :].rearrange(
                        "p (r s) -> p r s", r=2))

        def alloc_rows(pool, ktiles, n, tag, dt=BF):
            return [pool.tile([P, n], dt, tag=f"{tag}{k}", name=f"{tag}{k}")
                    for k in range(ktiles)]

        def fm_layer8(psum, src8, w8, pairs, mtiles, bias_sb, func, dst8):
            """fp8 DoubleRow feature-major layer into pair-packed fp8 dst."""
            for m in range(mtiles):
                pss = [psum.tile([P, NB], F32, tag="mm", name="mm")
                       for _ in range(NBLK)]
                for t in range(pairs):
                    lhs = w8[t][:, :, m * P:(m + 1) * P]
                    for tb in range(NBLK):
                        nc.tensor.matmul(pss[tb], lhs,
                                         src8[t][:, :, tb * NB:(tb + 1) * NB],
                                         start=(t == 0), stop=(t == pairs - 1),
                                         perf_mode=DR)
                for tb in range(NBLK):
                    dst = dst8[m // 2][:, m % 2, tb * NB:(tb + 1) * NB]
                    bias = bias_sb[:, m:m + 1] if bias_sb is not None else 0.0
                    nc.scalar.activation(dst, pss[tb], func,
                                         bias=bias, scale=1.0)

        # DRAM scratch (dependency-tracked tiles)
        svd, _free_svd = tc.tile([1, D], F32, space="DRAM", name="svd")
        cds, _free_cds = tc.tile([P, NT], F32, space="DRAM", name="cds")

        # ------ persistent attention operands (hm8, y8, v8) ------
        with tc.tile_pool(name="pq", bufs=1) as pq, \
             tc.tile_pool(name="py", bufs=1) as py, \
             tc.tile_pool(name="pvp", bufs=1) as pvp, \
             tc.tile_pool(name="pcl", bufs=1) as pcl:
            hm8 = alloc_pairs(pq, PD, S, "hm8")
            y8 = alloc_pairs(py, PD, S, "y8")
            v8 = alloc_pairs(pvp, NP, D, "v8")
            col8 = alloc_pairs(pcl, PD, 16, "col8")

            # ---------------- Stage A: x-MLP -> hm8 (in SBUF) ----------------
            with tc.tile_pool(name="wx", bufs=1) as wx, \
                 tc.tile_pool(name="px", bufs=1) as px, \
                 tc.tile_pool(name="phA", bufs=1) as phA, \
                 tc.tile_pool(name="phB", bufs=1) as phB, \
                 tc.tile_pool(name="psA", bufs=8, space="PSUM") as psA:
                x8 = alloc_pairs(px, PD, S, "x8")
                w18 = alloc_pairs(wx, PD, D, "w18")
                # first-needed tiles first: interleave x8 / W1 pair loads
                for t in range(PD):
                    nc.sync.dma_start(
                        out=x8[t], in_=x8d[t * P:(t + 1) * P, :].rearrange(
                            "p (r s) -> p r s", r=2))
                    nc.sync.dma_start(
                        out=w18[t], in_=W1d[t * P:(t + 1) * P, :].rearrange(
                            "p (r s) -> p r s", r=2))
                w28 = alloc_pairs(wx, PD, DB, "w28")
                load_pairs(w28, W2d, DB)
                w38 = alloc_pairs(wx, PB, D, "w38")
                load_pairs(w38, W3d, D)
                w48 = alloc_pairs(wx, PD, D, "w48")
                load_pairs(w48, W4d, D)
                m8 = alloc_pairs(wx, PD, D, "m8")
                load_pairs(m8, M8d, D)
                # y-side prefetch (queued behind stage A's needs)
                load_pairs(y8, y8d, S)
                load_pairs(col8, C8d, 16)

                h18 = alloc_pairs(phA, PD, S, "ha")
                h28 = alloc_pairs(phB, PB, S, "hb")
                h38 = alloc_pairs(phA, PD, S, "ha")   # reuse phA slots
                h48 = alloc_pairs(phB, PD, S, "hb")   # grow phB to 4 pair slots
                fm_layer8(psA, x8, w18, PD, KD, b1_sb, AF.Relu, h18)
                fm_layer8(psA, h18, w28, PD, KB, b2_sb, AF.Relu, h28)
                fm_layer8(psA, h28, w38, PB, KD, b3_sb, AF.Relu, h38)
                fm_layer8(psA, h38, w48, PD, KD, b4_sb, AF.Relu, h48)
                fm_layer8(psA, h48, m8, PD, KD, None, AF.Identity, hm8)

            # -------- Stage B: c bias, v8 (fp8), Sv0 (exact-ish) --------
            with tc.tile_pool(name="pwv", bufs=1) as pwv, \
                 tc.tile_pool(name="psBc", bufs=2, space="PSUM") as psBc, \
                 tc.tile_pool(name="psBv", bufs=2, space="PSUM") as psBv, \
                 tc.tile_pool(name="psBs", bufs=2, space="PSUM") as psBs:
                wv8 = alloc_pairs(pwv, PD, D, "wv8")
                load_pairs(wv8, Wv8d, D)
                # yT (bf16, for ysum) + Wv bf16 (for Sv0) hide under compute
                ys = alloc_rows(pwv, KD, S, "y")
                for k in range(KD):
                    nc.sync.dma_start(out=ys[k], in_=yTd[k * P:(k + 1) * P, :])
                wvs = alloc_rows(pwv, KD, D, "wv")
                for k in range(KD):
                    nc.sync.dma_start(out=wvs[k], in_=Wvbd[k * P:(k + 1) * P, :])

                # c_j = y_j . (Wk bq): [1, S] psum chunks -> SBUF row -> DRAM
                # (interleaved so the readback is a plain [128, 16] tile)
                csb = small.tile([1, S], F32, tag="csb", name="csb")
                for nb in range(NBLK):
                    psc = psBc.tile([1, NB], F32, tag="cc", name="cc")
                    for t in range(PD):
                        nc.tensor.matmul(psc, col8[t][:, :, 0:1],
                                         y8[t][:, :, nb * NB:(nb + 1) * NB],
                                         start=(t == 0), stop=(t == PD - 1),
                                         perf_mode=DR)
                    nc.scalar.activation(csb[0:1, nb * NB:(nb + 1) * NB], psc,
                                         AF.Identity, bias=0.0,
                                         scale=float(SCALE / CS))
                cdst = bass.AP(tensor=cds.tensor, offset=cds.offset,
                               ap=[[0, 1], [1, NT], [NT, P]])
                nc.sync.dma_start(out=cdst, in_=csb)
                nc.sync.dma_start(out=cbias, in_=cds)

                # v8: fp8 token-major v = y @ Wv (no bias; bv added at the end)
                for tq in range(NT):
                    pv = psBv.tile([P, D], F32, tag="vv", name="vv")
                    lhss = [y8[t][:, :, tq * P:(tq + 1) * P] for t in range(PD)]
                    for t in range(PD):
                        nc.tensor.matmul(pv[:, 0:NB], lhss[t],
                                         wv8[t][:, :, 0:NB],
                                         start=(t == 0), stop=(t == PD - 1),
                                         perf_mode=DR)
                        nc.tensor.matmul(pv[:, NB:D], lhss[t],
                                         wv8[t][:, :, NB:D],
                                         start=(t == 0), stop=(t == PD - 1),
                                         perf_mode=DR)
                    nc.scalar.activation(v8[tq // 2][:, tq % 2, :], pv,
                                         AF.Identity, bias=0.0, scale=1.0)

                # Sv0 = (sum_k y_k) @ Wv in bf16, broadcast via DRAM bounce
                ysumf = small.tile([P, KD], F32, tag="ysf", name="ysf")
                for k in range(KD):
                    nc.vector.tensor_reduce(ysumf[:, k:k + 1], ys[k],
                                            axis=mybir.AxisListType.X,
                                            op=ALU.add)
                ysumb = small.tile([P, KD], BF, tag="ysb", name="ysb")
                nc.scalar.activation(ysumb, ysumf, AF.Identity,
                                     bias=0.0, scale=1.0)
                svsb = small.tile([1, D], F32, tag="svsb", name="svsb")
                for db in range(2):
                    psv = psBs.tile([1, NB], F32, tag="sv", name="sv")
                    for k in range(KD):
                        nc.tensor.matmul(psv, ysumb[:, k:k + 1],
                                         wvs[k][:, db * NB:(db + 1) * NB],
                                         start=(k == 0), stop=(k == KD - 1))
                    nc.scalar.activation(svsb[0:1, db * NB:(db + 1) * NB], psv,
                                         AF.Identity, bias=0.0, scale=1.0)
                nc.sync.dma_start(out=svd, in_=svsb)
                sv_bcast = bass.AP(tensor=svd.tensor, offset=svd.offset,
                                   ap=[[0, P]] + list(svd.ap)[1:])
                nc.gpsimd.dma_start(out=sv_rep, in_=sv_bcast)

            # ---------------- Stage C: attention ----------------
            with tc.tile_pool(name="pE", bufs=2) as pE, \
                 tc.tile_pool(name="pEt", bufs=4) as pEt, \
                 tc.tile_pool(name="psCs", bufs=3, space="PSUM") as psCs, \
                 tc.tile_pool(name="psCo", bufs=2, space="PSUM") as psCo, \
                 tc.tile_pool(name="psCS", bufs=1, space="PSUM") as psCS:
                for half in range(2):
                    qoff = half * HALF
                    # Em1^T = exp(scale'*scores^T + c) - 1 in fp8 kv-pairs
                    ets8 = alloc_pairs(pE, NP, HALF, "e")
                    for tk in range(NT):
                        for qb in range(HALF // NB):
                            ps = psCs.tile([P, NB], F32, tag="sc", name="sc")
                            for t in range(PD):
                                nc.tensor.matmul(
                                    ps, y8[t][:, :, tk * P:(tk + 1) * P],
                                    hm8[t][:, :,
                                           qoff + qb * NB:qoff + (qb + 1) * NB],
                                    start=(t == 0), stop=(t == PD - 1),
                                    perf_mode=DR)
                            etmp = pEt.tile([P, NB], BF, tag="et", name="et")
                            nc.scalar.activation(etmp, ps, AF.Exp,
                                                 bias=cbias[:, tk:tk + 1],
                                                 scale=float(SCALE / MS))
                            nc.scalar.activation(
                                ets8[tk // 2][:, tk % 2, qb * NB:(qb + 1) * NB],
                                etmp, AF.Identity, bias=negone, scale=1.0)
                    # out rows: po = Em1 @ v8 (+Sv0), den = 2048 + Em1 @ 1
                    for tq8 in range(HALF // P):
                        tq = half * (HALF // P) + tq8
                        po = psCo.tile([P, D], F32, tag="oo", name="oo")
                        pS = psCS.tile([P, 16], F32, tag="ss", name="ss")
                        for t in range(NP):
                            lhs = ets8[t][:, :, tq8 * P:(tq8 + 1) * P]
                            nc.tensor.matmul(po[:, 0:NB], lhs, v8[t][:, :, 0:NB],
                                             start=(t == 0), stop=(t == NP - 1),
                                             perf_mode=DR)
                            nc.tensor.matmul(po[:, NB:D], lhs, v8[t][:, :, NB:D],
                                             start=(t == 0), stop=(t == NP - 1),
                                             perf_mode=DR)
                            nc.tensor.matmul(pS, lhs, ones8,
                                             start=(t == 0), stop=(t == NP - 1),
                                             perf_mode=DR)
                        dent = rpool.tile([P, 1], F32, tag="dd", name="dd")
                        nc.vector.tensor_scalar_add(dent, pS[:, 0:1], float(S))
                        rinv = rpool.tile([P, 1], F32, tag="ri", name="ri")
                        nc.vector.reciprocal(rinv, dent)
                        t1 = outp.tile([P, D], F32, tag="t1", name="t1")
                        nc.vector.tensor_add(t1, po, sv_rep)
                        ot = outp.tile([P, D], F32, tag="ot", name="ot")
                        nc.vector.scalar_tensor_tensor(
                            ot, t1, rinv, bv_rep, op0=ALU.mult, op1=ALU.add)
                        nc.sync.dma_start(out=out[tq * P:(tq + 1) * P, :],
                                          in_=ot)

        _free_svd()
        _free_cds()

    nc.compile()
    _NC = nc
    return nc


def _pack8(w):
    """[K, N] -> DoubleRow pair-packed fp8 [K/2, 2N]:
    out[t*128+p, r*N+m] = w[(2t+r)*128+p, m]."""
    K, N = w.shape
    return np.ascontiguousarray(
        w.astype(FP8).reshape(K // 256, 2, 128, N)
        .transpose(0, 2, 1, 3).reshape(K // 2, 2 * N))


def make_in_maps(inputs):
    """Host-side prep: per-core batch shard, weight folding (M = Wq Wk^T,
    col = Wk bq), fp8/bf16 casts + pair packing, feature-major transposes."""
    x = np.asarray(inputs["x"])
    y = np.asarray(inputs["y"])
    shared = {}
    for k in ("W1", "W2", "W3", "W4"):
        shared[k] = _pack8(np.asarray(inputs[k]).astype(np.float32))
    Wq = np.asarray(inputs["Wq"]).astype(np.float64)
    Wk = np.asarray(inputs["Wk"]).astype(np.float64)
    bq = np.asarray(inputs["bq"]).astype(np.float64)
    M = (Wq @ Wk.T).astype(np.float32)
    shared["M8"] = _pack8(M * MS)
    colm = np.zeros((D, 16), np.float32)
    colm[:, 0] = (Wk @ bq).astype(np.float32) * CS
    shared["C8"] = _pack8(colm)
    Wv = np.asarray(inputs["Wv"]).astype(np.float32)
    shared["Wv8"] = _pack8(Wv)
    shared["Wvb"] = np.ascontiguousarray(Wv.astype(BF16))
    for k, nt in (("b1", KD), ("b2", KB), ("b3", KD), ("b4", KD)):
        shared[k] = np.ascontiguousarray(
            np.asarray(inputs[k]).astype(np.float32).reshape(nt, P).T)
    shared["bv"] = np.ascontiguousarray(
        np.asarray(inputs["bv"]).astype(np.float32).reshape(D))
    in_maps = []
    for b in range(x.shape[0]):
        m = dict(shared)
        xT = np.ascontiguousarray(x[b].T)
        yT = np.ascontiguousarray(y[b].T)
        m["x8"] = _pack8(xT)
        m["y8"] = _pack8(yT)
        m["yT"] = yT.astype(BF16)
        in_maps.append(m)
    return in_maps


def kernel(**inputs):
    from concourse.bass_utils import run_bass_kernel_spmd

    nc = build_nc()
    in_maps = make_in_maps(inputs)
    res = run_bass_kernel_spmd(nc, in_maps, list(range(len(in_maps))))
    return np.stack([np.asarray(r["out"], dtype=np.float32)
                     for r in res.results])


# revision 12
# speedup vs baseline: 1.0014x; 1.0014x over previous
"""Trainium2 Bass kernel for nn_CrossAttention (4-layer MLP -> cross-attention).

Sharding: data-parallel across batch B=8, one batch element per NeuronCore.

v2 strategy — all-fp8 DoubleRow tensor work (PE was 92% busy in v1 with
~45% of its cycles in bf16):

  - Wq/Wk folded on host: M = Wq @ Wk^T, so scores = h @ M @ y^T and the
    k-projection disappears. The softmax-row-constant term h.(Wq bk)
    cancels; the column term c_j = y_j.(Wk bq) is a per-kv-partition bias
    applied for free inside the exp activation.
  - attention uses E@v = Sv + (E-1)@v: (E-1) in [-0.12, 0.12] quantizes to
    fp8 with ~16x less absolute error than E itself, and fp8 v errors only
    enter scaled by (E-1). Sv = (sum_k y_k) @ Wv is computed exactly-enough
    in bf16 (tiny matvec) and broadcast via a DRAM bounce. bv is added
    exactly in f32 at the very end: out = (po + Sv0)*rinv + bv.
  - v projection also runs fp8 DR from the already-resident y8 tiles.
  - M8 prescaled x32 on host (fp8 subnormal avoidance), compensated in the
    exp activation scale; col8 prescaled x8 likewise.

Remaining bf16: the 16 Sv matvec matmuls. All accumulation fp32 in PSUM.

fp8 operands are pair-packed for DoubleRow: logical contraction index
k = (2t+r)*128+p lives in tile t, partition p, middle index r, i.e. SBUF
tiles [128, 2, N] (pre-packed on host to [K/2, 2*N] per tile DMA).

Everything is SBUF-resident; y-side and attention operands prefetch early.
"""

import sys

if "/opt/trn_rl_repo" not in sys.path:
    sys.path.insert(0, "/opt/trn_rl_repo")

import numpy as np
import ml_dtypes

P = 128
D = 1024
DB = 512
S = 2048
KD = D // P       # 8 feature tiles of 128
KB = DB // P      # 4
PD = KD // 2      # 4 fp8 pair-tiles for a 1024 contraction
PB = KB // 2      # 2 for 512
NT = S // P       # 16 token tiles
NP = NT // 2      # 8 kv pair-tiles
NB = 512          # moving-operand free-dim block
NBLK = S // NB    # 4 token blocks
HALF = S // 2     # q processed in 2 halves during attention
NCORES = 8
SCALE = float(1.0 / np.sqrt(D))
MS = 32.0         # host prescale of M8 (fp8 subnormal avoidance)
CS = 8.0          # host prescale of col8

BF16 = ml_dtypes.bfloat16
FP8 = ml_dtypes.float8_e4m3

_NC = None


def build_nc():
    """Build + compile the per-core Bass program (cached)."""
    global _NC
    if _NC is not None:
        return _NC

    from contextlib import ExitStack
    import concourse.bass as bass
    import concourse.tile as tile
    from concourse import bacc, mybir

    BF = mybir.dt.bfloat16
    F8 = mybir.dt.float8e4
    F32 = mybir.dt.float32
    AF = mybir.ActivationFunctionType
    ALU = mybir.AluOpType
    DR = mybir.MatmulPerfMode.DoubleRow

    nc = bacc.Bacc("TRN2", target_bir_lowering=False, debug=False,
                   num_devices=NCORES)

    def din(name, shape, dt):
        return nc.dram_tensor(name, shape, dt, kind="ExternalInput").ap()

    # fp8 operands arrive pair-packed: [K/2, 2*N]
    x8d = din("x8", [D // 2, 2 * S], F8)
    y8d = din("y8", [D // 2, 2 * S], F8)
    yTd = din("yT", [D, S], BF)
    W1d = din("W1", [D // 2, 2 * D], F8)
    W2d = din("W2", [D // 2, 2 * DB], F8)
    W3d = din("W3", [DB // 2, 2 * D], F8)
    W4d = din("W4", [D // 2, 2 * D], F8)
    M8d = din("M8", [D // 2, 2 * D], F8)
    C8d = din("C8", [D // 2, 2 * 16], F8)
    Wv8d = din("Wv8", [D // 2, 2 * D], F8)
    Wvbd = din("Wvb", [D, D], BF)
    b1 = din("b1", [P, KD], F32)
    b2 = din("b2", [P, KB], F32)
    b3 = din("b3", [P, KD], F32)
    b4 = din("b4", [P, KD], F32)
    bv = din("bv", [D], F32)
    out = nc.dram_tensor("out", [S, D], F32, kind="ExternalOutput").ap()

    with tile.TileContext(nc) as tc, ExitStack() as ctx:
        small = ctx.enter_context(tc.tile_pool(name="small", bufs=1))
        rpool = ctx.enter_context(tc.tile_pool(name="rpool", bufs=4))
        outp = ctx.enter_context(tc.tile_pool(name="outp", bufs=2))

        def load_bias(src, cols, tag):
            t = small.tile([P, cols], F32, tag=tag, name=tag)
            nc.gpsimd.dma_start(out=t, in_=src)
            return t

        b1_sb = load_bias(b1, KD, "b1")
        b2_sb = load_bias(b2, KB, "b2")
        b3_sb = load_bias(b3, KD, "b3")
        b4_sb = load_bias(b4, KD, "b4")

        # bv replicated across partitions for the final (exact, fp32) bias add
        bv_rep = small.tile([P, D], F32, tag="bvrep", name="bvrep")
        bv_bcast = bass.AP(tensor=bv.tensor, offset=bv.offset,
                           ap=[[0, P]] + list(bv.ap))
        nc.gpsimd.dma_start(out=bv_rep, in_=bv_bcast)

        # fp8 all-ones moving operand for the denominator matmuls
        ones8 = small.tile([P, 2, 16], F8, tag="ones8", name="ones8")
        nc.vector.memset(ones8, 1.0)
        # per-partition -1.0 bias for the (E-1) activation
        negone = small.tile([P, 1], F32, tag="negone", name="negone")
        nc.vector.memset(negone, -1.0)

        # Sv0 replicated (filled in stage B via DRAM bounce)
        sv_rep = small.tile([P, D], F32, tag="svrep", name="svrep")
        # exp bias: SCALE * c_j per kv token, [128, 16] (col tk)
        cbias = small.tile([P, NT], F32, tag="cbias", name="cbias")

        def alloc_pairs(pool, pairs, n, tag, dt=F8):
            """fp8 pair-packed tiles [P, 2, n]."""
            return [pool.tile([P, 2, n], dt, tag=f"{tag}{t}", name=f"{tag}{t}")
                    for t in range(pairs)]

        def load_pairs(tiles, src, n):
            for t, tl in enumerate(tiles):
                nc.sync.dma_start(
                    out=tl,
                    in_=src[t * P:(t + 1) * P, :].rearrange(
                        "p (r s) -> p r s", r=2))

        def alloc_rows(pool, ktiles, n, tag, dt=BF):
            return [pool.tile([P, n], dt, tag=f"{tag}{k}", name=f"{tag}{k}")
                    for k in range(ktiles)]

        def fm_layer8(psum, src8, w8, pairs, mtiles, bias_sb, func, dst8):
            """fp8 DoubleRow feature-major layer into pair-packed fp8 dst."""
            for m in range(mtiles):
                pss = [psum.tile([P, NB], F32, tag="mm", name="mm")
                       for _ in range(NBLK)]
                for t in range(pairs):
                    lhs = w8[t][:, :, m * P:(m + 1) * P]
                    for tb in range(NBLK):
                        nc.tensor.matmul(pss[tb], lhs,
                                         src8[t][:, :, tb * NB:(tb + 1) * NB],
                                         start=(t == 0), stop=(t == pairs - 1),
                                         perf_mode=DR)
                for tb in range(NBLK):
                    dst = dst8[m // 2][:, m % 2, tb * NB:(tb + 1) * NB]
                    bias = bias_sb[:, m:m + 1] if bias_sb is not None else 0.0
                    nc.scalar.activation(dst, pss[tb], func,
                                         bias=bias, scale=1.0)

        # DRAM scratch (dependency-tracked tiles)
        svd, _free_svd = tc.tile([1, D], F32, space="DRAM", name="svd")
        cds, _free_cds = tc.tile([P, NT], F32, space="DRAM", name="cds")

        # ------ persistent attention operands (hm8, y8, v8) ------
        with tc.tile_pool(name="pq", bufs=1) as pq, \
             tc.tile_pool(name="py", bufs=1) as py, \
             tc.tile_pool(name="pvp", bufs=1) as pvp, \
             tc.tile_pool(name="pcl", bufs=1) as pcl:
            hm8 = alloc_pairs(pq, PD, S, "hm8")
            y8 = alloc_pairs(py, PD, S, "y8")
            v8 = alloc_pairs(pvp, NP, D, "v8")
            col8 = alloc_pairs(pcl, PD, 16, "col8")

            # ---------------- Stage A: x-MLP -> hm8 (in SBUF) ----------------
            with tc.tile_pool(name="wx", bufs=1) as wx, \
                 tc.tile_pool(name="px", bufs=1) as px, \
                 tc.tile_pool(name="phA", bufs=1) as phA, \
                 tc.tile_pool(name="phB", bufs=1) as phB, \
                 tc.tile_pool(name="psA", bufs=8, space="PSUM") as psA:
                x8 = alloc_pairs(px, PD, S, "x8")
                w18 = alloc_pairs(wx, PD, D, "w18")
                # first-needed tiles first: chunk the t=0 loads so the first
                # matmul can start early, then interleave x8 / W1 pair loads
                w1v0 = W1d[0:P, :].rearrange("p (r s) -> p r s", r=2)
                x8v0 = x8d[0:P, :].rearrange("p (r s) -> p r s", r=2)
                nc.sync.dma_start(out=w18[0][:, :, 0:P], in_=w1v0[:, :, 0:P])
                nc.sync.dma_start(out=x8[0][:, :, 0:NB], in_=x8v0[:, :, 0:NB])
                nc.sync.dma_start(out=w18[0][:, :, P:D], in_=w1v0[:, :, P:D])
                nc.sync.dma_start(out=x8[0][:, :, NB:S], in_=x8v0[:, :, NB:S])
                for t in range(1, PD):
                    nc.sync.dma_start(
                        out=x8[t], in_=x8d[t * P:(t + 1) * P, :].rearrange(
                            "p (r s) -> p r s", r=2))
                    nc.sync.dma_start(
                        out=w18[t], in_=W1d[t * P:(t + 1) * P, :].rearrange(
                            "p (r s) -> p r s", r=2))
                w28 = alloc_pairs(wx, PD, DB, "w28")
                load_pairs(w28, W2d, DB)
                w38 = alloc_pairs(wx, PB, D, "w38")
                load_pairs(w38, W3d, D)
                w48 = alloc_pairs(wx, PD, D, "w48")
                load_pairs(w48, W4d, D)
                m8 = alloc_pairs(wx, PD, D, "m8")
                load_pairs(m8, M8d, D)
                # y-side prefetch (queued behind stage A's needs)
                load_pairs(y8, y8d, S)
                load_pairs(col8, C8d, 16)

                h18 = alloc_pairs(phA, PD, S, "ha")
                h28 = alloc_pairs(phB, PB, S, "hb")
                h38 = alloc_pairs(phA, PD, S, "ha")   # reuse phA slots
                h48 = alloc_pairs(phB, PD, S, "hb")   # grow phB to 4 pair slots
                fm_layer8(psA, x8, w18, PD, KD, b1_sb, AF.Relu, h18)
                fm_layer8(psA, h18, w28, PD, KB, b2_sb, AF.Relu, h28)
                fm_layer8(psA, h28, w38, PB, KD, b3_sb, AF.Relu, h38)
                fm_layer8(psA, h38, w48, PD, KD, b4_sb, AF.Relu, h48)
                fm_layer8(psA, h48, m8, PD, KD, None, AF.Identity, hm8)

            # -------- Stage B: c bias, v8 (fp8), Sv0 (exact-ish) --------
            with tc.tile_pool(name="pwv", bufs=1) as pwv, \
                 tc.tile_pool(name="psBc", bufs=2, space="PSUM") as psBc, \
                 tc.tile_pool(name="psBv", bufs=2, space="PSUM") as psBv, \
                 tc.tile_pool(name="psBs", bufs=2, space="PSUM") as psBs:
                wv8 = alloc_pairs(pwv, PD, D, "wv8")
                load_pairs(wv8, Wv8d, D)
                # yT (bf16, for ysum) + Wv bf16 (for Sv0) hide under compute
                ys = alloc_rows(pwv, KD, S, "y")
                for k in range(KD):
                    nc.sync.dma_start(out=ys[k], in_=yTd[k * P:(k + 1) * P, :])
                wvs = alloc_rows(pwv, KD, D, "wv")
                for k in range(KD):
                    nc.sync.dma_start(out=wvs[k], in_=Wvbd[k * P:(k + 1) * P, :])

                # c_j = y_j . (Wk bq): [1, S] psum chunks -> SBUF row -> DRAM
                # (interleaved so the readback is a plain [128, 16] tile)
                csb = small.tile([1, S], F32, tag="csb", name="csb")
                for nb in range(NBLK):
                    psc = psBc.tile([1, NB], F32, tag="cc", name="cc")
                    for t in range(PD):
                        nc.tensor.matmul(psc, col8[t][:, :, 0:1],
                                         y8[t][:, :, nb * NB:(nb + 1) * NB],
                                         start=(t == 0), stop=(t == PD - 1),
                                         perf_mode=DR)
                    nc.scalar.activation(csb[0:1, nb * NB:(nb + 1) * NB], psc,
                                         AF.Identity, bias=0.0,
                                         scale=float(SCALE / CS))
                cdst = bass.AP(tensor=cds.tensor, offset=cds.offset,
                               ap=[[0, 1], [1, NT], [NT, P]])
                nc.sync.dma_start(out=cdst, in_=csb)
                nc.sync.dma_start(out=cbias, in_=cds)

                # v8: fp8 token-major v = y @ Wv (no bias; bv added at the end)
                for tq in range(NT):
                    pv = psBv.tile([P, D], F32, tag="vv", name="vv")
                    lhss = [y8[t][:, :, tq * P:(tq + 1) * P] for t in range(PD)]
                    for t in range(PD):
                        nc.tensor.matmul(pv[:, 0:NB], lhss[t],
                                         wv8[t][:, :, 0:NB],
                                         start=(t == 0), stop=(t == PD - 1),
                                         perf_mode=DR)
                        nc.tensor.matmul(pv[:, NB:D], lhss[t],
                                         wv8[t][:, :, NB:D],
                                         start=(t == 0), stop=(t == PD - 1),
                                         perf_mode=DR)
                    nc.scalar.activation(v8[tq // 2][:, tq % 2, :], pv,
                                         AF.Identity, bias=0.0, scale=1.0)

                # Sv0 = (sum_k y_k) @ Wv in bf16, broadcast via DRAM bounce
                ysumf = small.tile([P, KD], F32, tag="ysf", name="ysf")
                for k in range(KD):
                    nc.vector.tensor_reduce(ysumf[:, k:k + 1], ys[k],
                                            axis=mybir.AxisListType.X,
                                            op=ALU.add)
                ysumb = small.tile([P, KD], BF, tag="ysb", name="ysb")
                nc.scalar.activation(ysumb, ysumf, AF.Identity,
                                     bias=0.0, scale=1.0)
                svsb = small.tile([1, D], F32, tag="svsb", name="svsb")
                for db in range(2):
                    psv = psBs.tile([1, NB], F32, tag="sv", name="sv")
                    for k in range(KD):
                        nc.tensor.matmul(psv, ysumb[:, k:k + 1],
                                         wvs[k][:, db * NB:(db + 1) * NB],
                                         start=(k == 0), stop=(k == KD - 1))
                    nc.scalar.activation(svsb[0:1, db * NB:(db + 1) * NB], psv,
                                         AF.Identity, bias=0.0, scale=1.0)
                nc.sync.dma_start(out=svd, in_=svsb)
                sv_bcast = bass.AP(tensor=svd.tensor, offset=svd.offset,
                                   ap=[[0, P]] + list(svd.ap)[1:])
                nc.gpsimd.dma_start(out=sv_rep, in_=sv_bcast)

            # ---------------- Stage C: attention ----------------
            with tc.tile_pool(name="pE", bufs=2) as pE, \
                 tc.tile_pool(name="pEt", bufs=4) as pEt, \
                 tc.tile_pool(name="psCs", bufs=2, space="PSUM") as psCs, \
                 tc.tile_pool(name="psCo", bufs=2, space="PSUM") as psCo, \
                 tc.tile_pool(name="psCS", bufs=2, space="PSUM") as psCS:
                for half in range(2):
                    qoff = half * HALF
                    # Em1^T = exp(scale'*scores^T + c) - 1 in fp8 kv-pairs
                    ets8 = alloc_pairs(pE, NP, HALF, "e")
                    for tk in range(NT):
                        for qb in range(HALF // NB):
                            ps = psCs.tile([P, NB], F32, tag="sc", name="sc")
                            for t in range(PD):
                                nc.tensor.matmul(
                                    ps, y8[t][:, :, tk * P:(tk + 1) * P],
                                    hm8[t][:, :,
                                           qoff + qb * NB:qoff + (qb + 1) * NB],
                                    start=(t == 0), stop=(t == PD - 1),
                                    perf_mode=DR)
                            etmp = pEt.tile([P, NB], BF, tag="et", name="et")
                            nc.scalar.activation(etmp, ps, AF.Exp,
                                                 bias=cbias[:, tk:tk + 1],
                                                 scale=float(SCALE / MS))
                            nc.vector.tensor_scalar_add(
                                ets8[tk // 2][:, tk % 2, qb * NB:(qb + 1) * NB],
                                etmp, -1.0)
                    # out rows: po = Em1 @ v8 (+Sv0), den = 2048 + Em1 @ 1
                    for tq8 in range(HALF // P):
                        tq = half * (HALF // P) + tq8
                        po = psCo.tile([P, D], F32, tag="oo", name="oo")
                        pS = psCS.tile([P, 16], F32, tag="ss", name="ss")
                        for t in range(NP):
                            lhs = ets8[t][:, :, tq8 * P:(tq8 + 1) * P]
                            nc.tensor.matmul(po[:, 0:NB], lhs, v8[t][:, :, 0:NB],
                                             start=(t == 0), stop=(t == NP - 1),
                                             perf_mode=DR)
                            nc.tensor.matmul(po[:, NB:D], lhs, v8[t][:, :, NB:D],
                                             start=(t == 0), stop=(t == NP - 1),
                                             perf_mode=DR)
                            nc.tensor.matmul(pS, lhs, ones8,
                                             start=(t == 0), stop=(t == NP - 1),
                                             perf_mode=DR)
                        dent = rpool.tile([P, 1], F32, tag="dd", name="dd")
                        nc.vector.tensor_scalar_add(dent, pS[:, 0:1], float(S))
                        rinv = rpool.tile([P, 1], F32, tag="ri", name="ri")
                        nc.vector.reciprocal(rinv, dent)
                        t1 = outp.tile([P, D], F32, tag="t1", name="t1")
                        nc.vector.tensor_add(t1, po, sv_rep)
                        ot = outp.tile([P, D], F32, tag="ot", name="ot")
                        nc.vector.scalar_tensor_tensor(
                            ot, t1, rinv, bv_rep, op0=ALU.mult, op1=ALU.add)
                        nc.sync.dma_start(out=out[tq * P:(tq + 1) * P, :],
                                          in_=ot)

        _free_svd()
        _free_cds()

    nc.compile()
    _NC = nc
    return nc


def _pack8(w):
    """[K, N] -> DoubleRow pair-packed fp8 [K/2, 2N]:
    out[t*128+p, r*N+m] = w[(2t+r)*128+p, m]."""
    K, N = w.shape
    return np.ascontiguousarray(
        w.astype(FP8).reshape(K // 256, 2, 128, N)
        .transpose(0, 2, 1, 3).reshape(K // 2, 2 * N))


def make_in_maps(inputs):
    """Host-side prep: per-core batch shard, weight folding (M = Wq Wk^T,
    col = Wk bq), fp8/bf16 casts + pair packing, feature-major transposes."""
    x = np.asarray(inputs["x"])
    y = np.asarray(inputs["y"])
    shared = {}
    for k in ("W1", "W2", "W3", "W4"):
        shared[k] = _pack8(np.asarray(inputs[k]).astype(np.float32))
    Wq = np.asarray(inputs["Wq"]).astype(np.float64)
    Wk = np.asarray(inputs["Wk"]).astype(np.float64)
    bq = np.asarray(inputs["bq"]).astype(np.float64)
    M = (Wq @ Wk.T).astype(np.float32)
    shared["M8"] = _pack8(M * MS)
    colm = np.zeros((D, 16), np.float32)
    colm[:, 0] = (Wk @ bq).astype(np.float32) * CS
    shared["C8"] = _pack8(colm)
    Wv = np.asarray(inputs["Wv"]).astype(np.float32)
    shared["Wv8"] = _pack8(Wv)
    shared["Wvb"] = np.ascontiguousarray(Wv.astype(BF16))
    for k, nt in (("b1", KD), ("b2", KB), ("b3", KD), ("b4", KD)):
        shared[k] = np.ascontiguousarray(
            np.asarray(inputs[k]).astype(np.float32).reshape(nt, P).T)
    shared["bv"] = np.ascontiguousarray(
        np.asarray(inputs["bv"]).astype(np.float32).reshape(D))
    in_maps = []
    for b in range(x.shape[0]):
        m = dict(shared)
        xT = np.ascontiguousarray(x[b].T)
        yT = np.ascontiguousarray(y[b].T)
        m["x8"] = _pack8(xT)
        m["y8"] = _pack8(yT)
        m["yT"] = yT.astype(BF16)
        in_maps.append(m)
    return in_maps


def kernel(**inputs):
    from concourse.bass_utils import run_bass_kernel_spmd

    nc = build_nc()
    in_maps = make_in_maps(inputs)
    res = run_bass_kernel_spmd(nc, in_maps, list(range(len(in_maps))))
    return np.stack([np.asarray(r["out"], dtype=np.float32)
                     for r in res.results])
